# revision 1
# baseline (speedup 1.0000x reference)
"""Trainium2 Bass kernel for the BDH recurrent block (B=8, T=256, d=256, n=1024).

Key reformulation: the scan input v_prev is the *embedding* at each step (the
output v_star is never fed back), so the only recurrences are

  x_t = l1norm(0.97 * x_{t-1} + relu(emb_t @ Dx.T))          (elementwise, n)
  rho_t = 0.97 * rho_{t-1} + ln(emb_t) (x) x_t               (rank-1, d*n)

Both have closed forms:
  x_t  = sum_s C[t,s] * U_s           with U = relu(emb @ Dx.T)  and
         C[t,s] = 0.97^{t-s} / prod_{r=s..t} b_r,  b_r = sum(U_r) + 0.97*[r>0]
         (b_0 = sum(U_0)), computed in log space via a cumulative sum.
  a*_t = rho_{t-1} x_t = sum_{s<t} 0.97^{t-1-s} (x_s . x_t) ln(emb_s)
       = ((X X^T) o D) @ ln(emb)     -- decay-masked attention.

So the whole T-step scan becomes a handful of dense matmuls, one sample per
NeuronCore (data-parallel over B=8 across 8 cores, weights replicated).

This version runs the heavy matmuls in bf16 (operands quantized to bf16, f32
PSUM accumulation) which halves HBM traffic vs f32, and keeps the
precision-critical log-cumsum chain and all layernorm statistics in f32.
DMAs are packed into few large row-contiguous transfers issued in need-order
on both HWDGE queues (sync=SP, scalar=ACT).
"""

import numpy as np
import ml_dtypes

import concourse.bass as bass
import concourse.tile as tile
from concourse import bacc, mybir
from concourse.bass_utils import run_bass_kernel_spmd
from concourse.hw_specs import get_activation_tables

B, T, D, N = 8, 256, 256, 1024
P = 128  # partitions
LN_EPS = 1e-5
DECAY = 0.97
F32 = mybir.dt.float32
BF16 = mybir.dt.bfloat16
AF = mybir.ActivationFunctionType
ALU = mybir.AluOpType


def _build_nc():
    nc = bacc.Bacc(enable_partition_id=False)

    # dram tensors (packed; row slices are contiguous per-partition lines)
    # crit{k} = [embT half k | DxT half k]  (d on partitions) -- critical path
    d_crit0 = nc.dram_tensor("crit0", [P, T + N], BF16, kind="ExternalInput")
    d_crit1 = nc.dram_tensor("crit1", [P, T + N], BF16, kind="ExternalInput")
    d_sc = nc.dram_tensor("sc", [T, 4], F32, kind="ExternalInput")
    d_tid = nc.dram_tensor("tid", [P, 2 * P], F32, kind="ExternalInput")
    d_maskCT = nc.dram_tensor("maskCT", [T, T], F32, kind="ExternalInput")
    d_emb = nc.dram_tensor("emb", [T, D], F32, kind="ExternalInput")
    # late{k} = [DupT half k | DyT half k]
    d_late0 = nc.dram_tensor("late0", [P, T + N], BF16, kind="ExternalInput")
    d_late1 = nc.dram_tensor("late1", [P, T + N], BF16, kind="ExternalInput")
    # ET packed host-side as [p, k, d] -> [128, 2048]
    d_ET = nc.dram_tensor("ET", [P, 8 * D], BF16, kind="ExternalInput")
    # iotaP split hi/lo as two bf16 rows for the p broadcast matmul
    d_pk = nc.dram_tensor("pk", [2, T], BF16, kind="ExternalInput")
    d_out = nc.dram_tensor("out", [T, D], BF16, kind="ExternalOutput")

    # One ACT table set containing every function we use (relu/ln/exp/copy)
    # so the compiler never swaps tables mid-kernel (~2.7us per swap).
    act_sets = list(get_activation_tables(nc.m.arch))
    combined_set_id = act_sets.index("natural_log_exp_and_others")

    with tile.TileContext(nc) as tc:
        with (
            tc.tile_pool(name="consts", bufs=1) as cp,
            tc.tile_pool(name="work", bufs=1) as wp,
            tc.tile_pool(name="ps512", bufs=3, space="PSUM") as ps512,
            tc.tile_pool(name="ps256", bufs=4, space="PSUM") as ps256,
            tc.tile_pool(name="ps_small", bufs=1, space="PSUM") as pss,
        ):
            # ---- loads: issue order == need order, split over both queues ---
            crit = []
            for k, eng, dram in ((0, nc.sync, d_crit0), (1, nc.scalar, d_crit1)):
                t = cp.tile([P, T + N], BF16, tag=f"crit{k}", name=f"crit{k}")
                eng.dma_start(t[:], dram[:, :])
                crit.append(t)
            embT_s = [t[:, 0:T] for t in crit]
            DxT_s = [t[:, T:T + N] for t in crit]

            # act table load on scalar right after its critical dma issue
            nc.scalar.add_instruction(mybir.InstLoadActFuncSet(
                name=nc.get_next_instruction_name(),
                act_func_set_id=combined_set_id, ins=[], outs=[]))

            sc_t = cp.tile([P, 2, 4], F32, tag="sc", name="sc")
            nc.sync.dma_start(sc_t[:], d_sc.rearrange("(m p) c -> p m c", p=P))
            c097_s = [sc_t[:, m, 0:1] for m in range(2)]
            iotaP_s = [sc_t[:, m, 1:2] for m in range(2)]
            iotaQ_s = [sc_t[:, m, 2:3] for m in range(2)]

            tid_s = cp.tile([P, 2 * P], F32, tag="tid", name="tid")
            nc.sync.dma_start(tid_s[:], d_tid[:, :])
            triu_s = tid_s[:, 0:P]
            ident_s = tid_s[:, P:2 * P]

            maskCT_s = []
            for k, eng in ((0, nc.sync), (1, nc.scalar)):
                t = cp.tile([P, T], F32, tag=f"maskCT{k}", name=f"maskCT{k}")
                eng.dma_start(t[:], d_maskCT[k * P:(k + 1) * P, :])
                maskCT_s.append(t)

            emb_s = []
            for k, eng in ((0, nc.sync), (1, nc.scalar)):
                t = cp.tile([P, D], F32, tag=f"emb{k}", name=f"emb{k}")
                eng.dma_start(t[:], d_emb[k * P:(k + 1) * P, :])
                emb_s.append(t)

            late = []
            for k, eng, dram in ((0, nc.sync, d_late0), (1, nc.scalar, d_late1)):
                t = cp.tile([P, T + N], BF16, tag=f"late{k}", name=f"late{k}")
                eng.dma_start(t[:], dram[:, :])
                late.append(t)
            DupT_s = [t[:, 0:T] for t in late]
            DyT_s = [t[:, T:T + N] for t in late]

            et_big = cp.tile([P, 8, D], BF16, tag="et_big", name="et_big")
            nc.sync.dma_start(et_big[:, 0:4, :], d_ET[:, 0:4 * D])
            nc.scalar.dma_start(et_big[:, 4:8, :], d_ET[:, 4 * D:8 * D])
            ET_s = [et_big[:, k, :] for k in range(8)]

            # bf16 triu/ones for the (split) log-space cumsum matmuls
            tri2 = cp.tile([P, 2, P], BF16, tag="tri2", name="tri2")
            nc.vector.tensor_copy(tri2[:, 0, :], triu_s)
            nc.vector.memset(tri2[:, 1, :], 1.0)
            # inclusive-cumsum row blocks for the transposed (row) cumsum:
            # block 0 = [triu_incl | ones], block 1 = [zeros | triu_incl]
            cumA = cp.tile([P, 2, T], BF16, tag="cumA", name="cumA")
            ti_f = wp.tile([P, P], F32, tag="ti_f", name="ti_f")
            nc.vector.tensor_add(ti_f[:], triu_s, ident_s)
            nc.vector.tensor_copy(cumA[:, 0, 0:P], ti_f[:])
            nc.vector.memset(cumA[:, 0, P:T], 1.0)
            nc.vector.memset(cumA[:, 1, 0:P], 0.0)
            nc.vector.tensor_copy(cumA[:, 1, P:T], ti_f[:])
            # p-broadcast operands: iotaP rows + a tiny ones lhsT
            pk_t = cp.tile([2, T], BF16, tag="pk", name="pk")
            nc.sync.dma_start(pk_t[:], d_pk[:, :])
            ones2 = cp.tile([2, P], BF16, tag="ones2", name="ones2")
            nc.vector.memset(ones2[:], 1.0)
            zero_col = cp.tile([P, 1], F32, tag="zero_col", name="zero_col")
            nc.vector.memset(zero_col[:], 0.0)
            eps_col = cp.tile([P, 1], F32, tag="eps_col", name="eps_col")
            nc.vector.memset(eps_col[:], LN_EPS)

            # ---- U = relu(emb @ Dx.T), row sums a ---------------------------
            U_s = [wp.tile([P, N], BF16, tag=f"U{m}", name=f"U{m}")
                   for m in range(2)]
            apart2 = wp.tile([P, 2, 2], F32, tag="apart2", name="apart2")
            for mt in range(2):
                for ch in range(2):
                    pu = ps512.tile([P, 512], F32, tag="pu", name="pu")
                    for k in range(2):
                        nc.tensor.matmul(
                            pu[:], embT_s[k][:, mt * P:(mt + 1) * P],
                            DxT_s[k][:, ch * 512:(ch + 1) * 512],
                            start=(k == 0), stop=(k == 1))
                    if ch == 0:
                        nc.scalar.activation(
                            out=U_s[mt][:, ch * 512:(ch + 1) * 512], in_=pu[:],
                            func=AF.Relu, bias=zero_col[:],
                            accum_out=apart2[:, ch, mt:mt + 1])
                    else:
                        nc.vector.tensor_scalar(
                            U_s[mt][:, ch * 512:(ch + 1) * 512], pu[:], 0.0,
                            0.0, op0=ALU.max, op1=ALU.add,
                            accum_out=apart2[:, ch, mt:mt + 1])

            # Critical chain (U sums -> cumsum -> p/q -> CT) gets top
            # scheduler priority so engine-idle slots don't fill with
            # deferrable work ahead of it.  All per-step vectors are kept
            # [P,2]-wide (both halves of T at once) to halve DVE op count;
            # logb is split hi/lo into bf16 so the cumsum matmuls and the
            # p broadcast run as single-pass bf16 PE ops (f32 PSUM
            # recombination keeps ~2^-18 relative accuracy).
            with tc.high_priority():
                # b = a0 + a1 + 0.97*[t>0]; logb = ln(b)
                bcol = wp.tile([P, 2], F32, tag="bcol", name="bcol")
                nc.vector.tensor_add(bcol[:], apart2[:, 0, :], apart2[:, 1, :])
                nc.vector.tensor_add(bcol[:], bcol[:], sc_t[:, :, 0])
                lbcol = wp.tile([P, 2], F32, tag="lbcol", name="lbcol")
                nc.scalar.activation(out=lbcol[:], in_=bcol[:], func=AF.Ln,
                                     bias=zero_col[:])
                lb2 = wp.tile([P, 2, 2], BF16, tag="lb2", name="lb2")
                nc.vector.tensor_copy(lb2[:, :, 0], lbcol[:])
                lbl = wp.tile([P, 2], F32, tag="lbl", name="lbl")
                nc.vector.tensor_sub(lbl[:], lbcol[:], lb2[:, :, 0])
                nc.vector.tensor_copy(lb2[:, :, 1], lbl[:])

                # column (strict) cumsum for q_s: hi+lo accumulate in
                # PSUM across split matmuls (exact f32, no DVE reduce)
                qcol = wp.tile([P, 2], F32, tag="qcol", name="qcol")
                for mt in range(2):
                    pl1 = pss.tile([P, 1], F32, tag="pss", name="plam")
                    if mt == 0:
                        mms = [(0, 0, 0), (0, 0, 1)]
                    else:
                        mms = [(1, 0, 0), (1, 0, 1), (0, 1, 0), (0, 1, 1)]
                    for i, (blk, m2, j) in enumerate(mms):
                        nc.tensor.matmul(pl1[:], tri2[:, blk, :],
                                         lb2[:, m2, j:j + 1],
                                         start=(i == 0),
                                         stop=(i == len(mms) - 1))
                    nc.vector.tensor_add(qcol[:, mt:mt + 1], pl1[:],
                                         sc_t[:, mt, 2:3])

                # pb[s,t] = p_t = iotaP_t - incsum_t, computed directly on
                # the PE: replicate the negated logb splits across columns
                # (cheap DVE ops, exact in bf16) and accumulate four
                # inclusive-cumsum matmuls plus the iotaP rows in f32 PSUM.
                reps = []
                for mt in range(2):
                    for j, srccol in ((0, lbcol), (1, lbl)):
                        rr = wp.tile([P, P], BF16, tag=f"rep{mt}{j}",
                                     name=f"rep{mt}{j}")
                        nc.vector.tensor_scalar(
                            rr[:], tri2[:, 1, :], srccol[:, mt:mt + 1], -1.0,
                            op0=ALU.mult, op1=ALU.mult)
                        reps.append((mt, rr))
                pb = ps256.tile([P, T], F32, tag="ps", name="pb")
                for i, (mt, rr) in enumerate(reps):
                    nc.tensor.matmul(pb[:], rr[:], cumA[:, mt, :],
                                     start=(i == 0), stop=False)
                nc.tensor.matmul(pb[:], ones2[:], pk_t[:], start=False,
                                 stop=True)

                # CT[s,t] = exp(q_s + p_t + mask): STT folds q + mask per st,
                # then a single wide exp over both halves.
                tmp2 = wp.tile([P, 2, T], F32, tag="ctmp", name="ctmp")
                nc.vector.scalar_tensor_tensor(
                    out=tmp2[:, 0, :], in0=pb[:], scalar=qcol[:, 0:1],
                    in1=maskCT_s[0][:], op0=ALU.add, op1=ALU.add)
                nc.vector.scalar_tensor_tensor(
                    out=tmp2[:, 1, :], in0=pb[:], scalar=qcol[:, 1:2],
                    in1=maskCT_s[1][:], op0=ALU.add, op1=ALU.add)
                ct2 = wp.tile([P, 2, T], BF16, tag="CT", name="CT")
                for st in range(2):
                    nc.scalar.activation(out=ct2[:, st, :],
                                         in_=tmp2[:, st, :], func=AF.Exp,
                                         bias=zero_col[:], scale=1.0)
                CT_s = [ct2[:, 0, :], ct2[:, 1, :]]

            # ---- X^T = U^T C^T  (n on partitions, T free) -------------------
            XT_s = []
            for m in range(8):
                px = ps256.tile([P, T], F32, tag="ps", name="px")
                for k in range(2):
                    nc.tensor.matmul(px[:], U_s[k][:, m * P:(m + 1) * P],
                                     CT_s[k][:], start=(k == 0), stop=(k == 1))
                xt = wp.tile([P, T], BF16, tag=f"XT{m}", name=f"XT{m}")
                if m % 2 == 0:
                    nc.vector.tensor_copy(xt[:], px[:])
                else:
                    nc.scalar.copy(xt[:], px[:])
                XT_s.append(xt)

            # ---- W = ln(emb rows) -------------------------------------------
            W_s = []
            if True:  # (keep block indent; scheduler-sensitive region)
              for mt in range(2):
                st6 = wp.tile([P, 6], F32, tag=f"wst{mt}", name=f"wst{mt}")
                nc.vector.bn_stats(st6[:], emb_s[mt][:])
                mv = wp.tile([P, 2], F32, tag=f"wmv{mt}", name=f"wmv{mt}")
                nc.vector.bn_aggr(mv[:], st6[:])
                lv = wp.tile([P, 1], F32, tag=f"wlv{mt}", name=f"wlv{mt}")
                nc.scalar.activation(out=lv[:], in_=mv[:, 1:2], func=AF.Ln,
                                     bias=eps_col[:])
                rs = wp.tile([P, 1], F32, tag=f"wrs{mt}", name=f"wrs{mt}")
                nc.scalar.activation(out=rs[:], in_=lv[:], func=AF.Exp,
                                     bias=zero_col[:], scale=-0.5)
                w = wp.tile([P, D], BF16, tag=f"W{mt}", name=f"W{mt}")
                nc.vector.tensor_scalar(w[:], emb_s[mt][:], mv[:, 0:1],
                                        rs[:], op0=ALU.subtract, op1=ALU.mult)
                W_s.append(w)

            # ---- G = X X^T ; GD = G o Dup -----------------------------------
            # DupT[s,t] = decay^(t-1-s) for s<t else 0, so the st=1 row block
            # only needs columns t >= 128; its left half is identically zero.
            GD_s = []
            for st in range(2):
                cols = slice(0, T) if st == 0 else slice(P, T)
                pg = ps256.tile([P, T], F32, tag="ps", name="pg")
                for k in range(8):
                    nc.tensor.matmul(pg[:, cols],
                                     XT_s[k][:, st * P:(st + 1) * P],
                                     XT_s[k][:, cols],
                                     start=(k == 0), stop=(k == 7))
                gd = wp.tile([P, T], BF16, tag=f"GD{st}", name=f"GD{st}")
                if st == 1:
                    nc.vector.memset(gd[:, 0:P], 0.0)
                nc.vector.tensor_mul(gd[:, cols], pg[:, cols],
                                     DupT_s[st][:, cols])
                GD_s.append(gd)

            # ---- A = (G o D) @ W  ([t, d]) + layernorm ----------------------
            # mt=1 first: its longer LN chain then overlaps mt=0's matmuls.
            Aln_s = [None, None]
            for mt in (1, 0):
                pa = ps256.tile([P, D], F32, tag="ps", name="pa")
                ks = [0] if mt == 0 else [0, 1]
                for k in ks:
                    nc.tensor.matmul(pa[:], GD_s[k][:, mt * P:(mt + 1) * P],
                                     W_s[k][:], start=(k == ks[0]),
                                     stop=(k == ks[-1]))
                st6 = wp.tile([P, 6], F32, tag=f"ast{mt}", name=f"ast{mt}")
                nc.vector.bn_stats(st6[:], pa[:])
                mv = wp.tile([P, 2], F32, tag=f"amv{mt}", name=f"amv{mt}")
                nc.vector.bn_aggr(mv[:], st6[:])
                lv = wp.tile([P, 1], F32, tag=f"alv{mt}", name=f"alv{mt}")
                nc.scalar.activation(out=lv[:], in_=mv[:, 1:2], func=AF.Ln,
                                     bias=eps_col[:])
                rs = wp.tile([P, 1], F32, tag=f"ars{mt}", name=f"ars{mt}")
                nc.scalar.activation(out=rs[:], in_=lv[:], func=AF.Exp,
                                     bias=zero_col[:], scale=-0.5)
                al = wp.tile([P, D], F32, tag=f"Aln{mt}", name=f"Aln{mt}")
                nc.vector.tensor_scalar(al[:], pa[:], mv[:, 0:1], rs[:],
                                        op0=ALU.subtract, op1=ALU.mult)
                Aln_s[mt] = al

            # ---- Aln^T via PE transpose ([d, t]), cast to bf16 --------------
            AlnT_s = [wp.tile([P, T], BF16, tag=f"AlnT{k}", name=f"AlnT{k}")
                      for k in range(2)]
            for mt, dt_ in ((0, 0), (0, 1), (1, 0), (1, 1)):
                    ptr = ps256.tile([P, P], F32, tag="ps", name="atr")
                    nc.tensor.transpose(
                        ptr[:], Aln_s[mt][:, dt_ * P:(dt_ + 1) * P], ident_s[:])
                    nc.vector.tensor_copy(
                        AlnT_s[dt_][:, mt * P:(mt + 1) * P], ptr[:])

            # ---- y^T = relu(Dy ln(A)^T) o X^T -------------------------------
            yT_s = []
            for m in range(8):
                py = ps256.tile([P, T], F32, tag="ps", name="py")
                for k in range(2):
                    nc.tensor.matmul(py[:], DyT_s[k][:, m * P:(m + 1) * P],
                                     AlnT_s[k][:], start=(k == 0),
                                     stop=(k == 1))
                yt = wp.tile([P, T], BF16, tag=f"yT{m}", name=f"yT{m}")
                nc.vector.scalar_tensor_tensor(
                    out=yt[:], in0=py[:], scalar=0.0, in1=XT_s[m][:],
                    op0=ALU.max, op1=ALU.mult)
                yT_s.append(yt)

            # ---- v = y E^T ([t, d]) + layernorm + store ---------------------
            for mt in range(2):
                pv = ps256.tile([P, D], F32, tag="ps", name="pv")
                for k in range(8):
                    nc.tensor.matmul(pv[:], yT_s[k][:, mt * P:(mt + 1) * P],
                                     ET_s[k][:], start=(k == 0), stop=(k == 7))
                st6 = wp.tile([P, 6], F32, tag=f"ost{mt}", name=f"ost{mt}")
                nc.vector.bn_stats(st6[:], pv[:])
                mv = wp.tile([P, 2], F32, tag=f"omv{mt}", name=f"omv{mt}")
                nc.vector.bn_aggr(mv[:], st6[:])
                lv = wp.tile([P, 1], F32, tag=f"olv{mt}", name=f"olv{mt}")
                nc.scalar.activation(out=lv[:], in_=mv[:, 1:2], func=AF.Ln,
                                     bias=eps_col[:])
                rs = wp.tile([P, 1], F32, tag=f"ors{mt}", name=f"ors{mt}")
                nc.scalar.activation(out=rs[:], in_=lv[:], func=AF.Exp,
                                     bias=zero_col[:], scale=-0.5)
                ov = wp.tile([P, D], BF16, tag=f"ov{mt}", name=f"ov{mt}")
                nc.vector.tensor_scalar(ov[:], pv[:], mv[:, 0:1], rs[:],
                                        op0=ALU.subtract, op1=ALU.mult)
                nc.sync.dma_start(d_out[mt * P:(mt + 1) * P, :], ov[:])

    nc.finalize()
    return nc


_NC_CACHE = {}


def _get_nc(_unused=True):
    if "nc" not in _NC_CACHE:
        _NC_CACHE["nc"] = _build_nc()
    return _NC_CACHE["nc"]


def _host_consts():
    ii = np.arange(T, dtype=np.float64)
    ln097 = np.log(np.float64(DECAY))
    maskCT = np.where(ii[:, None] <= ii[None, :], 0.0, -1e30).astype(np.float32)
    DupT = np.where(
        ii[:, None] < ii[None, :],
        np.float64(DECAY) ** (ii[None, :] - 1 - ii[:, None]),
        0.0,
    ).astype(np.float32)
    tid = np.ascontiguousarray(np.concatenate(
        [np.triu(np.ones((P, P), np.float32), k=1), np.eye(P, dtype=np.float32)],
        axis=1))
    sc = np.zeros((T, 4), np.float32)
    sc[:, 0] = DECAY
    sc[0, 0] = 0.0
    sc[:, 1] = (ii * ln097).astype(np.float32)
    sc[:, 2] = (-ii * ln097).astype(np.float32)
    bf = ml_dtypes.bfloat16
    iotaP = (ii * ln097).astype(np.float32)
    p_hi = iotaP.astype(bf)
    p_lo = (iotaP - p_hi.astype(np.float32)).astype(bf)
    pk = np.stack([p_hi, p_lo], axis=0)  # [2, T] bf16
    return sc, tid, maskCT, DupT, pk


def make_in_maps(embeddings, E, Dx, Dy):
    bf = ml_dtypes.bfloat16
    emb = np.ascontiguousarray(np.asarray(embeddings, dtype=np.float32))
    E = np.asarray(E, dtype=np.float32)
    Dx = np.asarray(Dx, dtype=np.float32)
    Dy = np.asarray(Dy, dtype=np.float32)
    sc, tid, maskCT, DupT, pk = _host_consts()

    DxT = np.ascontiguousarray(Dx.T).astype(bf)      # [D, N]
    DyT = np.ascontiguousarray(Dy.T).astype(bf)      # [D, N]
    ETp = np.ascontiguousarray(                      # [P, 8*D]
        E.T.reshape(8, P, D).transpose(1, 0, 2).reshape(P, 8 * D)).astype(bf)
    DupT_bf = DupT.astype(bf)

    shared = {
        "sc": sc, "tid": tid, "maskCT": maskCT, "ET": ETp, "pk": pk,
        "late0": np.ascontiguousarray(
            np.concatenate([DupT_bf[0:P], DyT[0:P]], axis=1)),
        "late1": np.ascontiguousarray(
            np.concatenate([DupT_bf[P:T], DyT[P:T]], axis=1)),
    }

    in_maps = []
    for b in range(B):
        embT_bf = np.ascontiguousarray(emb[b].T).astype(bf)  # [D, T]
        m = dict(shared)
        m["crit0"] = np.ascontiguousarray(
            np.concatenate([embT_bf[0:P], DxT[0:P]], axis=1))
        m["crit1"] = np.ascontiguousarray(
            np.concatenate([embT_bf[P:D], DxT[P:D]], axis=1))
        m["emb"] = emb[b]
        in_maps.append(m)
    return in_maps


def kernel(embeddings, E, Dx, Dy, **_kw):
    in_maps = make_in_maps(embeddings, E, Dx, Dy)
    nc = _get_nc()
    res = run_bass_kernel_spmd(nc, in_maps, core_ids=list(range(B)))
    return np.stack([np.asarray(r["out"]).astype(np.float32)
                     for r in res.results], axis=0)



# revision 5
# speedup vs baseline: 1.0926x; 1.0926x over previous
"""Trainium2 Bass kernel for the BDH recurrent block (B=8, T=256, d=256, n=1024).

Key reformulation: the scan input v_prev is the *embedding* at each step (the
output v_star is never fed back), so the only recurrences are

  x_t = l1norm(0.97 * x_{t-1} + relu(emb_t @ Dx.T))          (elementwise, n)
  rho_t = 0.97 * rho_{t-1} + ln(emb_t) (x) x_t               (rank-1, d*n)

Both have closed forms:
  x_t  = sum_s C[t,s] * U_s           with U = relu(emb @ Dx.T)  and
         C[t,s] = 0.97^{t-s} / prod_{r=s..t} b_r,  b_r = sum(U_r) + 0.97*[r>0]
         (b_0 = sum(U_0)), computed in log space via a cumulative sum.
  a*_t = rho_{t-1} x_t = sum_{s<t} 0.97^{t-1-s} (x_s . x_t) ln(emb_s)
       = ((X X^T) o D) @ ln(emb)     -- decay-masked attention.

So the whole T-step scan becomes a handful of dense matmuls, one sample per
NeuronCore (data-parallel over B=8 across 8 cores, weights replicated).

v2 structural optimizations over the first working version:
 *  ln(a*) is never computed.  W rows are exactly zero-mean, so a* rows are
    zero-mean (mu_t = 0), and the per-row 1/sigma of ln(a*) commutes with relu
    and is absorbed by the final row-wise layernorm up to an exact eps
    correction:  out_t = (z - zbar) * rsqrt(var(z) + eps*(va_t + eps)) with
    va_t = var_d(a*_t).  va_t is computed off the critical path from squared
    A^T columns via tiny ones-matmuls.  This deletes the LN(A) chain and all
    four PE transposes: A^T = W^T @ (G o D) is computed directly.
 *  The causal -1e30 mask is accumulated into the p-broadcast PSUM banks via
    constant matmuls (strict-triangular x negated-identity), so CT is a single
    Exp activation per half reading PSUM with bias=q, no DVE op in between,
    and the maskCT DMA disappears.
 *  PE warm-up: the PE clock is HAM-gated at 1.2 GHz until it has been busy
    ~3.4us, and re-throttles after ~3.4us idle.  Junk matmuls on a zeroed tile
    cover the initial DMA wait, and low-priority filler matmuls let the
    scheduler plug remaining idle windows, keeping the array at 2.4 GHz.
 *  Critical input DMAs are split finer across both HWDGE queues and emb is
    loaded in bf16.
"""

import numpy as np
import ml_dtypes

import concourse.bass as bass
import concourse.tile as tile
from concourse import bacc, mybir
from concourse.bass_utils import run_bass_kernel_spmd
from concourse.hw_specs import get_activation_tables

B, T, D, N = 8, 256, 256, 1024
P = 128  # partitions
LN_EPS = 1e-5
DECAY = 0.97
NEGBIG = -1e30
F32 = mybir.dt.float32
BF16 = mybir.dt.bfloat16
AF = mybir.ActivationFunctionType
ALU = mybir.AluOpType

N_JUNK_START = 12   # warm-up matmuls (free dim 256) before the first U matmul
N_JUNK_FILL = 10    # low-priority filler matmuls placed into idle PE windows


def _build_nc():
    nc = bacc.Bacc(enable_partition_id=False)

    # dram tensors.  critXa = [embT half X | DxT half X cols 0:512],
    # critXb = [DxT half X cols 512:1024] -- split so the first U matmuls
    # start as soon as the first 192KB lands.
    d_c0a = nc.dram_tensor("c0a", [P, T + 512], BF16, kind="ExternalInput")
    d_c0b = nc.dram_tensor("c0b", [P, 512], BF16, kind="ExternalInput")
    d_c1a = nc.dram_tensor("c1a", [P, T + 512], BF16, kind="ExternalInput")
    d_c1b = nc.dram_tensor("c1b", [P, 512], BF16, kind="ExternalInput")
    d_sc = nc.dram_tensor("sc", [T, 4], F32, kind="ExternalInput")
    # consts = [tri_strict(128) | ones(128) | cumA0(256) | cumA1(256) |
    #           negI0(256) | negI1(256)]  (bf16, [128, 1280])
    d_consts = nc.dram_tensor("consts", [P, 1280], BF16, kind="ExternalInput")
    d_emb = nc.dram_tensor("emb", [T, D], BF16, kind="ExternalInput")
    # late{k} = [DupT half k | DyT half k]
    d_late0 = nc.dram_tensor("late0", [P, T + N], BF16, kind="ExternalInput")
    d_late1 = nc.dram_tensor("late1", [P, T + N], BF16, kind="ExternalInput")
    # ET packed host-side as [p, k, d] -> [128, 2048]
    d_ET = nc.dram_tensor("ET", [P, 8 * D], BF16, kind="ExternalInput")
    # iotaP split hi/lo as two bf16 rows for the p broadcast matmul
    d_pk = nc.dram_tensor("pk", [2, T], BF16, kind="ExternalInput")
    d_out = nc.dram_tensor("out", [T, D], BF16, kind="ExternalOutput")

    # One ACT table set containing every function we use
    # (relu/ln/exp/copy/identity) so the compiler never swaps tables.
    act_sets = list(get_activation_tables(nc.m.arch))
    combined_set_id = act_sets.index("natural_log_exp_and_others")

    with tile.TileContext(nc) as tc:
        with (
            tc.tile_pool(name="consts", bufs=1) as cp,
            tc.tile_pool(name="work", bufs=1) as wp,
            tc.tile_pool(name="ps512", bufs=3, space="PSUM") as ps512,
            tc.tile_pool(name="ps256", bufs=2, space="PSUM") as ps256,
            tc.tile_pool(name="ps_pbm", bufs=1, space="PSUM") as pbp,
            tc.tile_pool(name="ps_small", bufs=1, space="PSUM") as pss,
        ):
            # ---- loads: issue order == need order, both queues -------------
            c0a = cp.tile([P, T + 512], BF16, tag="c0a", name="c0a")
            nc.sync.dma_start(c0a[:], d_c0a[:, :])
            c1a = cp.tile([P, T + 512], BF16, tag="c1a", name="c1a")
            nc.scalar.dma_start(c1a[:], d_c1a[:, :])

            # act table load on scalar right after its critical dma issue
            nc.scalar.add_instruction(mybir.InstLoadActFuncSet(
                name=nc.get_next_instruction_name(),
                act_func_set_id=combined_set_id, ins=[], outs=[]))

            c0b = cp.tile([P, 512], BF16, tag="c0b", name="c0b")
            nc.sync.dma_start(c0b[:], d_c0b[:, :])
            c1b = cp.tile([P, 512], BF16, tag="c1b", name="c1b")
            nc.scalar.dma_start(c1b[:], d_c1b[:, :])

            embT_s = [c0a[:, 0:T], c1a[:, 0:T]]
            # DxT column-halves: ch0 in critXa, ch1 in critXb
            DxT_a = [c0a[:, T:T + 512], c1a[:, T:T + 512]]
            DxT_b = [c0b[:], c1b[:]]

            sc_t = cp.tile([P, 2, 4], F32, tag="sc", name="sc")
            nc.sync.dma_start(sc_t[:], d_sc.rearrange("(m p) c -> p m c", p=P))
            iotaQ_s = [sc_t[:, m, 2:3] for m in range(2)]

            consts = cp.tile([P, 1280], BF16, tag="consts", name="consts")
            nc.sync.dma_start(consts[:], d_consts[:, :])
            tri_st = consts[:, 0:P]          # strict upper tri [k < c]
            ones_t = consts[:, P:2 * P]
            cumA = [consts[:, 256:512], consts[:, 512:768]]
            negI = [consts[:, 768:1024], consts[:, 1024:1280]]

            emb_s = []
            for k in range(2):
                t = cp.tile([P, D], BF16, tag=f"emb{k}", name=f"emb{k}")
                nc.scalar.dma_start(t[:], d_emb[k * P:(k + 1) * P, :])
                emb_s.append(t)

            late = []
            for k, eng, dram in ((0, nc.sync, d_late0), (1, nc.scalar, d_late1)):
                t = cp.tile([P, T + N], BF16, tag=f"late{k}", name=f"late{k}")
                eng.dma_start(t[:], dram[:, :])
                late.append(t)
            DupT_s = [t[:, 0:T] for t in late]
            DyT_s = [t[:, T:T + N] for t in late]

            et_big = cp.tile([P, 8, D], BF16, tag="et_big", name="et_big")
            nc.sync.dma_start(et_big[:, 0:4, :], d_ET[:, 0:4 * D])
            nc.scalar.dma_start(et_big[:, 4:8, :], d_ET[:, 4 * D:8 * D])
            ET_s = [et_big[:, k, :] for k in range(8)]

            pk_t = cp.tile([2, T], BF16, tag="pk", name="pk")
            nc.sync.dma_start(pk_t[:], d_pk[:, :])

            # ---- memset-built small consts ---------------------------------
            ones2 = cp.tile([2, P], BF16, tag="ones2", name="ones2")
            nc.vector.memset(ones2[:], 1.0)
            onescol = cp.tile([P, 1], BF16, tag="onescol", name="onescol")
            nc.vector.memset(onescol[:], 1.0)
            junk_sb = cp.tile([P, 512], BF16, tag="junk", name="junk")
            nc.vector.memset(junk_sb[:], 0.0)
            zero_col = cp.tile([P, 1], F32, tag="zero_col", name="zero_col")
            nc.vector.memset(zero_col[:], 0.0)
            eps_col = cp.tile([P, 1], F32, tag="eps_col", name="eps_col")
            nc.vector.memset(eps_col[:], LN_EPS)

            # ---- PE warm-up: junk matmuls covering the initial DMA wait ----
            for j in range(N_JUNK_START):
                jp = ps512.tile([P, 512], F32, tag="pu", name=f"jstart{j}")
                nc.tensor.matmul(jp[:, 0:256], junk_sb[:, 0:P],
                                 junk_sb[:, 0:256], start=True, stop=True)

            # ---- U = relu(emb @ Dx.T), row sums a ---------------------------
            U_s = [wp.tile([P, N], BF16, tag=f"U{m}", name=f"U{m}")
                   for m in range(2)]
            apart2 = wp.tile([P, 2, 2], F32, tag="apart2", name="apart2")
            for ch in range(2):           # ch0 first: both row-sum halves asap
                for mt in range(2):
                    pu = ps512.tile([P, 512], F32, tag="pu", name="pu")
                    for k in range(2):
                        dx = DxT_a[k] if ch == 0 else DxT_b[k]
                        nc.tensor.matmul(
                            pu[:], embT_s[k][:, mt * P:(mt + 1) * P],
                            dx[:], start=(k == 0), stop=(k == 1))
                    if ch == 0:
                        nc.scalar.activation(
                            out=U_s[mt][:, ch * 512:(ch + 1) * 512], in_=pu[:],
                            func=AF.Relu, bias=zero_col[:],
                            accum_out=apart2[:, ch, mt:mt + 1])
                    else:
                        nc.vector.tensor_scalar(
                            U_s[mt][:, ch * 512:(ch + 1) * 512], pu[:], 0.0,
                            0.0, op0=ALU.max, op1=ALU.add,
                            accum_out=apart2[:, ch, mt:mt + 1])

            # ---- mask + iotaP accumulated into the two p-broadcast banks ---
            # pbm[st][s,t] = p_t - 1e30*[t < s + 128*st]  (+ cumsum terms below)
            pbm = [pbp.tile([P, T], F32, tag=f"pbm{st}", name=f"pbm{st}")
                   for st in range(2)]
            nc.tensor.matmul(pbm[0][:], tri_st, negI[0], start=True, stop=False)
            nc.tensor.matmul(pbm[1][:], ones_t, negI[0], start=True, stop=False)
            nc.tensor.matmul(pbm[1][:], tri_st, negI[1], start=False, stop=False)
            for st in range(2):
                nc.tensor.matmul(pbm[st][:], ones2[:], pk_t[:], start=False,
                                 stop=False)

            # Critical chain (U sums -> logb -> hi/lo split -> cumsum matmuls
            # -> exp) gets top scheduler priority.
            with tc.high_priority():
                # b = a0 + a1 + 0.97*[t>0]; logb = ln(b)
                bcol = wp.tile([P, 2], F32, tag="bcol", name="bcol")
                nc.vector.tensor_add(bcol[:], apart2[:, 0, :], apart2[:, 1, :])
                nc.vector.tensor_add(bcol[:], bcol[:], sc_t[:, :, 0])
                lbcol = wp.tile([P, 2], F32, tag="lbcol", name="lbcol")
                nc.scalar.activation(out=lbcol[:], in_=bcol[:], func=AF.Ln,
                                     bias=zero_col[:])

                # hi/lo split of logb, replicated across 128 columns: hi via
                # ACT Identity broadcast (bias), lo via DVE tensor_scalar.
                rr = {}
                lbl = wp.tile([P, 2], F32, tag="lbl", name="lbl")
                for mt in range(2):
                    rh = wp.tile([P, P], BF16, tag=f"rrh{mt}", name=f"rrh{mt}")
                    nc.scalar.activation(out=rh[:], in_=junk_sb[:, 0:P],
                                         func=AF.Identity,
                                         bias=lbcol[:, mt:mt + 1], scale=0.0)
                    rr[(mt, 0)] = rh
                    nc.vector.tensor_sub(lbl[:, mt:mt + 1],
                                         lbcol[:, mt:mt + 1], rh[:, 0:1])
                    rl = wp.tile([P, P], BF16, tag=f"rrl{mt}", name=f"rrl{mt}")
                    nc.vector.tensor_scalar(rl[:], junk_sb[:, 0:P], 0.0,
                                            lbl[:, mt:mt + 1],
                                            op0=ALU.mult, op1=ALU.add)
                    rr[(mt, 1)] = rl

                # column (strict) cumsum for q_s in PSUM (exact f32)
                qps = pss.tile([P, 2], F32, tag="pss", name="qps")
                for mt in range(2):
                    if mt == 0:
                        mms = [(tri_st, 0, 0), (tri_st, 0, 1)]
                    else:
                        mms = [(ones_t, 0, 0), (ones_t, 0, 1),
                               (tri_st, 1, 0), (tri_st, 1, 1)]
                    for i, (lhs, m2, j) in enumerate(mms):
                        nc.tensor.matmul(qps[:, mt:mt + 1], lhs,
                                         rr[(m2, j)][:, 0:1],
                                         start=(i == 0),
                                         stop=(i == len(mms) - 1))
                qsb = wp.tile([P, 2], F32, tag="qsb", name="qsb")
                nc.vector.tensor_add(qsb[:], qps[:], sc_t[:, :, 2])

                # p-broadcast cumsum into both masked banks; negation is baked
                # into cumA (host sends -1/0 inclusive-cumsum blocks).
                ct2 = wp.tile([P, 2, T], BF16, tag="CT", name="CT")
                for st in range(2):
                    for i, (mt, j) in enumerate(
                            ((0, 0), (0, 1), (1, 0), (1, 1))):
                        nc.tensor.matmul(pbm[st][:], rr[(mt, j)][:],
                                         cumA[mt][:], start=False,
                                         stop=(i == 3))
                    # CT[s,t] = exp(q_s + p_t + mask) straight from PSUM
                    nc.scalar.activation(out=ct2[:, st, :], in_=pbm[st][:],
                                         func=AF.Exp,
                                         bias=qsb[:, st:st + 1], scale=1.0)
                CT_s = [ct2[:, 0, :], ct2[:, 1, :]]

            # ---- X^T = U^T C^T  (n on partitions, T free), 512-wide pairs --
            XT_p = []
            for jp in range(4):
                px = ps512.tile([P, 512], F32, tag="pu", name="px")
                for h in range(2):
                    m = 2 * jp + h
                    for k in range(2):
                        nc.tensor.matmul(px[:, h * T:(h + 1) * T],
                                         U_s[k][:, m * P:(m + 1) * P],
                                         CT_s[k][:], start=(k == 0),
                                         stop=(k == 1))
                xt = wp.tile([P, 512], BF16, tag=f"XT{jp}", name=f"XT{jp}")
                if jp % 2 == 0:
                    nc.vector.tensor_copy(xt[:], px[:])
                else:
                    nc.scalar.copy(xt[:], px[:])
                XT_p.append(xt)

            def xs(k, lo, hi):  # slice [lo:hi] of n-chunk k from the pairs
                return XT_p[k // 2][:, (k % 2) * T + lo:(k % 2) * T + hi]

            # ---- W = ln(emb rows) -------------------------------------------
            W_s = []
            for mt in range(2):
                st6 = wp.tile([P, 6], F32, tag=f"wst{mt}", name=f"wst{mt}")
                nc.vector.bn_stats(st6[:], emb_s[mt][:])
                mv = wp.tile([P, 2], F32, tag=f"wmv{mt}", name=f"wmv{mt}")
                nc.vector.bn_aggr(mv[:], st6[:])
                lv = wp.tile([P, 1], F32, tag=f"wlv{mt}", name=f"wlv{mt}")
                nc.scalar.activation(out=lv[:], in_=mv[:, 1:2], func=AF.Ln,
                                     bias=eps_col[:])
                rs = wp.tile([P, 1], F32, tag=f"wrs{mt}", name=f"wrs{mt}")
                nc.scalar.activation(out=rs[:], in_=lv[:], func=AF.Exp,
                                     bias=zero_col[:], scale=-0.5)
                w = wp.tile([P, D], BF16, tag=f"W{mt}", name=f"W{mt}")
                nc.vector.tensor_scalar(w[:], emb_s[mt][:], mv[:, 0:1],
                                        rs[:], op0=ALU.subtract, op1=ALU.mult)
                W_s.append(w)

            # ---- G = X X^T ; GD = G o Dup -----------------------------------
            # DupT[s,t] = decay^(t-1-s) for s<t else 0, so the st=1 row block
            # only needs columns t >= 128; its left half is identically zero.
            GD_s = []
            for st in range(2):
                cols = slice(0, T) if st == 0 else slice(P, T)
                pg = ps256.tile([P, T], F32, tag="ps", name="pg")
                for k in range(8):
                    nc.tensor.matmul(pg[:, cols],
                                     xs(k, st * P, (st + 1) * P),
                                     xs(k, cols.start, cols.stop),
                                     start=(k == 0), stop=(k == 7))
                gd = wp.tile([P, T], BF16, tag=f"GD{st}", name=f"GD{st}")
                if st == 1:
                    nc.vector.memset(gd[:, 0:P], 0.0)
                nc.vector.tensor_mul(gd[:, cols], pg[:, cols],
                                     DupT_s[st][:, cols])
                GD_s.append(gd)

            # ---- A^T = W^T (G o D)  ([d, t]), no layernorm needed -----------
            AT_s = []
            for dt_ in range(2):
                pa = ps256.tile([P, T], F32, tag="ps", name="pa")
                for k in range(2):
                    nc.tensor.matmul(pa[:], W_s[k][:, dt_ * P:(dt_ + 1) * P],
                                     GD_s[k][:], start=(k == 0), stop=(k == 1))
                at = wp.tile([P, T], BF16, tag=f"AT{dt_}", name=f"AT{dt_}")
                if dt_ == 0:
                    nc.vector.tensor_copy(at[:], pa[:])
                else:
                    nc.scalar.copy(at[:], pa[:])
                AT_s.append(at)

            # ---- va_t = sum_d a*[t,d]^2 -> per-row eps for the final LN ----
            # (exact compensation for the dropped 1/sigma of ln(a*):
            #  eps_t = LN_EPS * (va_t/D + LN_EPS))
            sq_s = []
            for k in range(2):
                sq = wp.tile([P, T], BF16, tag=f"sq{k}", name=f"sq{k}")
                nc.vector.tensor_mul(sq[:], AT_s[k][:], AT_s[k][:])
                sq_s.append(sq)
            epsva = []
            for mt in range(2):
                vap = pss.tile([P, 1], F32, tag="pss", name=f"va{mt}")
                for k in range(2):
                    nc.tensor.matmul(vap[:], sq_s[k][:, mt * P:(mt + 1) * P],
                                     onescol[:], start=(k == 0), stop=(k == 1))
                ev = wp.tile([P, 1], F32, tag=f"ev{mt}", name=f"ev{mt}")
                nc.vector.tensor_scalar(ev[:], vap[:], LN_EPS / D,
                                        LN_EPS * LN_EPS,
                                        op0=ALU.mult, op1=ALU.add)
                epsva.append(ev)

            # ---- y^T = relu(Dy A^T) o X^T, 512-wide pairs -------------------
            yT_p = []
            for jp in range(4):
                py = ps512.tile([P, 512], F32, tag="pu", name="py")
                for h in range(2):
                    m = 2 * jp + h
                    for k in range(2):
                        nc.tensor.matmul(py[:, h * T:(h + 1) * T],
                                         DyT_s[k][:, m * P:(m + 1) * P],
                                         AT_s[k][:], start=(k == 0),
                                         stop=(k == 1))
                yt = wp.tile([P, 512], BF16, tag=f"yT{jp}", name=f"yT{jp}")
                nc.vector.scalar_tensor_tensor(
                    out=yt[:], in0=py[:], scalar=0.0, in1=XT_p[jp][:],
                    op0=ALU.max, op1=ALU.mult)
                yT_p.append(yt)

            def ys(k, lo, hi):
                return yT_p[k // 2][:, (k % 2) * T + lo:(k % 2) * T + hi]

            # ---- v = y E^T ([t, d]) + layernorm (per-row eps) + store -------
            for mt in range(2):
                pv = ps256.tile([P, D], F32, tag="ps", name="pv")
                for k in range(8):
                    nc.tensor.matmul(pv[:], ys(k, mt * P, (mt + 1) * P),
                                     ET_s[k][:], start=(k == 0), stop=(k == 7))
                st6 = wp.tile([P, 6], F32, tag=f"ost{mt}", name=f"ost{mt}")
                nc.vector.bn_stats(st6[:], pv[:])
                mv = wp.tile([P, 2], F32, tag=f"omv{mt}", name=f"omv{mt}")
                nc.vector.bn_aggr(mv[:], st6[:])
                lv = wp.tile([P, 1], F32, tag=f"olv{mt}", name=f"olv{mt}")
                nc.scalar.activation(out=lv[:], in_=mv[:, 1:2], func=AF.Ln,
                                     bias=epsva[mt][:])
                rs = wp.tile([P, 1], F32, tag=f"ors{mt}", name=f"ors{mt}")
                nc.scalar.activation(out=rs[:], in_=lv[:], func=AF.Exp,
                                     bias=zero_col[:], scale=-0.5)
                ov = wp.tile([P, D], BF16, tag=f"ov{mt}", name=f"ov{mt}")
                nc.vector.tensor_scalar(ov[:], pv[:], mv[:, 0:1], rs[:],
                                        op0=ALU.subtract, op1=ALU.mult)
                nc.sync.dma_start(d_out[mt * P:(mt + 1) * P, :], ov[:])

            # ---- low-priority filler matmuls: scheduler drops these into
            # idle PE windows (e.g. while the log-cumsum chain runs) to keep
            # the HAM clock gate open.
            for j in range(N_JUNK_FILL):
                jp = ps512.tile([P, 512], F32, tag="pu", name=f"jfill{j}")
                nc.tensor.matmul(jp[:], junk_sb[:, 0:P], junk_sb[:],
                                 start=True, stop=True)

    nc.finalize()
    return nc


_NC_CACHE = {}


def _get_nc(_unused=True):
    if "nc" not in _NC_CACHE:
        _NC_CACHE["nc"] = _build_nc()
    return _NC_CACHE["nc"]


def _host_consts():
    bf = ml_dtypes.bfloat16
    ii = np.arange(T, dtype=np.float64)
    ln097 = np.log(np.float64(DECAY))
    DupT = np.where(
        ii[:, None] < ii[None, :],
        np.float64(DECAY) ** (ii[None, :] - 1 - ii[:, None]),
        0.0,
    ).astype(np.float32)
    sc = np.zeros((T, 4), np.float32)
    sc[:, 0] = DECAY
    sc[0, 0] = 0.0
    sc[:, 1] = (ii * ln097).astype(np.float32)
    sc[:, 2] = (-ii * ln097).astype(np.float32)

    tri_strict = np.triu(np.ones((P, P), np.float32), k=1)
    ones = np.ones((P, P), np.float32)
    incl = np.triu(np.ones((P, P), np.float32), k=0)
    zeros = np.zeros((P, P), np.float32)
    # cumA carries the NEGATIVE inclusive-cumsum blocks (the p_t term is
    # -sum_{r<=t} logb_r, and the rr broadcasts are positive).
    cumA0 = np.concatenate([-incl, -ones], axis=1)
    cumA1 = np.concatenate([zeros, -incl], axis=1)
    negI0 = np.concatenate([NEGBIG * np.eye(P, dtype=np.float32), zeros],
                           axis=1)
    negI1 = np.concatenate([zeros, NEGBIG * np.eye(P, dtype=np.float32)],
                           axis=1)
    consts = np.concatenate(
        [tri_strict, ones, cumA0, cumA1, negI0, negI1], axis=1).astype(bf)

    iotaP = (ii * ln097).astype(np.float32)
    p_hi = iotaP.astype(bf)
    p_lo = (iotaP - p_hi.astype(np.float32)).astype(bf)
    pk = np.stack([p_hi, p_lo], axis=0)  # [2, T] bf16
    return sc, consts, DupT.astype(bf), pk


def make_in_maps(embeddings, E, Dx, Dy):
    bf = ml_dtypes.bfloat16
    emb = np.ascontiguousarray(np.asarray(embeddings, dtype=np.float32))
    E = np.asarray(E, dtype=np.float32)
    Dx = np.asarray(Dx, dtype=np.float32)
    Dy = np.asarray(Dy, dtype=np.float32)
    sc, consts, DupT_bf, pk = _host_consts()

    DxT = np.ascontiguousarray(Dx.T).astype(bf)      # [D, N]
    DyT = np.ascontiguousarray(Dy.T).astype(bf)      # [D, N]
    ETp = np.ascontiguousarray(                      # [P, 8*D]
        E.T.reshape(8, P, D).transpose(1, 0, 2).reshape(P, 8 * D)).astype(bf)

    shared = {
        "sc": sc, "consts": consts, "ET": ETp, "pk": pk,
        "late0": np.ascontiguousarray(
            np.concatenate([DupT_bf[0:P], DyT[0:P]], axis=1)),
        "late1": np.ascontiguousarray(
            np.concatenate([DupT_bf[P:T], DyT[P:T]], axis=1)),
    }

    in_maps = []
    for b in range(B):
        embT_bf = np.ascontiguousarray(emb[b].T).astype(bf)  # [D, T]
        m = dict(shared)
        m["c0a"] = np.ascontiguousarray(
            np.concatenate([embT_bf[0:P], DxT[0:P, 0:512]], axis=1))
        m["c0b"] = np.ascontiguousarray(DxT[0:P, 512:1024])
        m["c1a"] = np.ascontiguousarray(
            np.concatenate([embT_bf[P:D], DxT[P:D, 0:512]], axis=1))
        m["c1b"] = np.ascontiguousarray(DxT[P:D, 512:1024])
        m["emb"] = emb[b].astype(bf)
        in_maps.append(m)
    return in_maps


def kernel(embeddings, E, Dx, Dy, **_kw):
    in_maps = make_in_maps(embeddings, E, Dx, Dy)
    nc = _get_nc()
    res = run_bass_kernel_spmd(nc, in_maps, core_ids=list(range(B)))
    return np.stack([np.asarray(r["out"]).astype(np.float32)
                     for r in res.results], axis=0)
